# revision 29
# baseline (speedup 1.0000x reference)
"""Trainium2 Bass kernel for BatchNorm2d + 8-head self-attention block.

Reference (per batch element b, all fp32):
    xn = BN_eval(x[b]); t = xn.T
    q/k/v = t @ W.T + b            # [S, 512], 8 heads x 64
    attn  = softmax(q k^T / 8)     # per head
    y[b]  = ((attn v) @ wo.T + bo).T

Sharding: pure data parallel — one batch element per NeuronCore, weights
replicated, no collectives.

Device design (per core), fully in the "transposed" domain (no large
transposes anywhere):
  - BN folded into QKV weights/biases on host; 1/8 scale folded into wq/bq;
    v bias folded into bo (softmax rows sum to 1).
  - Q^T,K^T [I,S] = wT.T @ x      (x arrives [C,S] — natural rhs)
  - V [S,I]       = x_chunk.T @ wvT, stored interleaved per head with a
    ones column ([128, 8*65]) so the PV matmul (M=65) also produces the
    softmax denominators for free.
  - scores^T per head [t,s]; head pairs row-packed via tile_position
    (0,0)/(64,0), K=64 each; exp on ScalarE over both heads in one call
    (no max subtraction — scores are in [-3, 3]).
  - o^T accumulates over 8 t-chunks (K=128); normalize = approx-reciprocal
    row + gpsimd partition-broadcast + DVE multiply; y^T = woT.T @ o^T + bo.

Matmul dtype is fp16 by default: 2-byte weights keep LDWEIGHTS in the PE's
background buffer (hidden behind the previous matmul) where 4-byte fp32r
weights serialize ~150ns per matmul; fp16's 10-bit mantissa keeps the end
to-end error ~1e-3 of scale (all activations are within [-20, 20]).
Set dt_mm=float32r for a ~1e-4-accuracy variant (~1.4x slower).
"""

import numpy as np

import concourse.bass as bass
import concourse.tile as tile
from concourse import bacc, mybir
from concourse.bass_utils import run_bass_kernel_spmd
from concourse.tile import add_dep_helper

B, C, S = 8, 512, 1024
H, DH, INNER = 8, 64, 512
EPS = 1e-5
SCALE = DH ** (-0.5)
N_CORES = 8
F32 = mybir.dt.float32
F32R = mybir.dt.float32r
F16 = mybir.dt.float16

DT_MM = F16  # matmul dtype: F16 (fast) or F32R (precise)

_CACHE: dict = {}

KC = C // 128      # 4 contraction chunks over channels
IT = INNER // 128  # 4 tiles over inner dim (also head-pair index)
ST = S // 128      # 8 t-chunks
NSLAB = S // 512   # 2 s-slabs


def build_bass(dt_mm):
    two_byte = mybir.dt.size(dt_mm) == 2
    dt_in = dt_mm if two_byte else F32
    nc = bacc.Bacc("TRN2", target_bir_lowering=False, debug=False,
                   num_devices=N_CORES)

    # inputs arrive pre-arranged on the host into the SBUF layout
    # [partition, k-chunk, free] so every DMA is contiguous per partition
    x_d = nc.dram_tensor("x", [128, KC, S], dt_in, kind="ExternalInput")
    wqT_d = nc.dram_tensor("wqT", [128, KC, 512], dt_in, kind="ExternalInput")
    wkT_d = nc.dram_tensor("wkT", [128, KC, 512], dt_in, kind="ExternalInput")
    wvT_d = nc.dram_tensor("wvT", [128, KC, 512], dt_in, kind="ExternalInput")
    woT_d = nc.dram_tensor("woT", [128, KC, 512], dt_in, kind="ExternalInput")
    # bq | bk | bo packed on host as [128, 12] (col t+0/4/8 = vec[t*128+p])
    bias_d = nc.dram_tensor("bias_pack", [128, 3 * IT], F32, kind="ExternalInput")
    y_d = nc.dram_tensor("y", [C, S], F32, kind="ExternalOutput")

    with tile.TileContext(nc) as tc:
        with (
            tc.tile_pool(name="persist", bufs=1) as persist,
            tc.tile_pool(name="stage", bufs=2) as stage,
            tc.tile_pool(name="out", bufs=3) as outp,
            tc.tile_pool(name="et", bufs=6) as etp,
            tc.tile_pool(name="norm", bufs=2) as normp,
            # one shared 4-slot pool for every 1-bank accumulator (projection
            # groups AND the two attention po accumulators): a fresh bank is
            # always available at head-pair transitions, so the strict-FIFO
            # PE queue never stalls behind the normalize chain.
            tc.tile_pool(name="psA", bufs=4, space="PSUM") as psA,
            tc.tile_pool(name="psS", bufs=2, space="PSUM") as psS,
        ):
            # ---- loads, chained on the sync/HWDGE queue in need-order so
            # bandwidth goes to the critical transfer instead of round-robin;
            # tiny bias pack rides the gpsimd/SWDGE queue in parallel ----
            xr = persist.tile([128, KC, S], dt_mm, tag="xr", name="xr")
            wqr = persist.tile([128, KC, 512], dt_mm, tag="wqr", name="wqr")
            wkr = persist.tile([128, KC, 512], dt_mm, tag="wkr", name="wkr")
            wvr = persist.tile([128, KC, 512], dt_mm, tag="wvr", name="wvr")
            wor = persist.tile([128, KC, 512], dt_mm, tag="wor", name="wor")

            dmas = []
            if two_byte:
                # DMA straight into the matmul tiles — no casts needed.
                for h in range(2):
                    dmas.append(nc.sync.dma_start(
                        xr[:, 2 * h:2 * h + 2, :], x_d[:, 2 * h:2 * h + 2, :]))
                for dst, src in ((wqr, wqT_d), (wkr, wkT_d), (wvr, wvT_d),
                                 (wor, woT_d)):
                    dmas.append(nc.sync.dma_start(dst[:], src[:]))
            else:
                x_st = [stage.tile([128, 2, S], F32, tag=f"x_st{k}",
                                   name=f"x_st{k}", bufs=1) for k in range(2)]
                for h in range(2):
                    dmas.append(nc.sync.dma_start(
                        x_st[h][:], x_d[:, 2 * h:2 * h + 2, :]))
                w_st = {}
                for nm, src in (("wq", wqT_d), ("wk", wkT_d), ("wv", wvT_d),
                                ("wo", woT_d)):
                    st = stage.tile([128, KC, 512], F32, tag=f"{nm}_st",
                                    name=f"{nm}_st", bufs=1)
                    w_st[nm] = st
                    dmas.append(nc.sync.dma_start(st[:], src[:]))
            for a, b in zip(dmas[1:], dmas):
                add_dep_helper(a.ins, b.ins, sync=False, reason="dma priority")

            bias_sb = persist.tile([128, 3 * IT], F32, tag="bias")
            nc.gpsimd.dma_start(bias_sb[:], bias_d[:])
            bq_sb = bias_sb[:, 0:IT]
            bk_sb = bias_sb[:, IT:2 * IT]
            bo_sb = bias_sb[:, 2 * IT:3 * IT]

            if not two_byte:
                for kc in range(KC):
                    nc.vector.tensor_copy(xr[:, kc, :], x_st[kc // 2][:, kc % 2, :])
                nc.vector.tensor_copy(wqr[:], w_st["wq"][:])
                nc.vector.tensor_copy(wkr[:], w_st["wk"][:])
                nc.vector.tensor_copy(wvr[:], w_st["wv"][:])
                nc.vector.tensor_copy(wor[:], w_st["wo"][:])

            ones_sb = persist.tile([128, H], F32, tag="ones")
            nc.vector.memset(ones_sb[:], 1.0)

            # ---- PE warmup: ~4us of dummy matmuls during the initial DMA
            # wait so the HAM clock-gate reaches 8/8 before real work ----
            warm_sb = stage.tile([128, 256], dt_mm, tag="warm", bufs=1)
            nc.vector.memset(warm_sb[:], 0.0)
            warm_ps = psA.tile([128, 256], F32, tag="acc", name="warm_ps")
            for wi in range(34):
                nc.tensor.matmul(warm_ps[:], warm_sb[:, 0:128], warm_sb[:],
                                 start=(wi == 0), stop=(wi == 33))

            # ---- persistent per-slab outputs ----
            qT = [[persist.tile([128, 512], dt_mm, tag=f"qT{i}{s}",
                                name=f"qT{i}{s}") for s in range(NSLAB)]
                  for i in range(IT)]
            kT = [[persist.tile([128, 512], dt_mm, tag=f"kT{i}{s}",
                                name=f"kT{i}{s}") for s in range(NSLAB)]
                  for i in range(IT)]
            oT = [[persist.tile([128, 512], dt_mm, tag=f"oT{i}{s}",
                                name=f"oT{i}{s}") for s in range(NSLAB)]
                  for i in range(IT)]
            v_sb = [persist.tile([128, H * 65], dt_mm, tag=f"v{t}",
                                 name=f"v{t}") for t in range(ST)]

            def group_thunks(n_mm, emit_mm, evac):
                """n_mm single-matmul thunks accumulating into one psA bank;
                the first allocates the bank, the last appends the evacuation."""
                box = []

                def mk(i):
                    def t():
                        if i == 0:
                            box.append(psA.tile([128, 512], F32,
                                                tag="acc", name="acc"))
                        emit_mm(box[0], i)
                        if i == n_mm - 1:
                            evac(box[0])
                    return t

                return [mk(i) for i in range(n_mm)]

            def qk_thunks(w, bias, dst, hp, sl):
                def emit_mm(ps, kc):
                    nc.tensor.matmul(
                        ps[:],
                        w[:, kc, hp * 128:(hp + 1) * 128],
                        xr[:, kc, sl * 512:(sl + 1) * 512],
                        start=(kc == 0), stop=(kc == KC - 1),
                    )

                def evac(ps):
                    nc.vector.tensor_scalar_add(
                        dst[hp][sl][:], ps[:], bias[:, hp:hp + 1]
                    )

                return group_thunks(KC, emit_mm, evac)

            def v_thunks(tc_):
                def emit_mm(ps, kc):
                    nc.tensor.matmul(
                        ps[:],
                        xr[:, kc, tc_ * 128:(tc_ + 1) * 128],
                        wvr[:, kc, :],
                        start=(kc == 0), stop=(kc == KC - 1),
                    )

                def evac(ps):
                    vv = v_sb[tc_][:].rearrange("p (h m) -> p h m", h=H)
                    nc.vector.tensor_copy(
                        vv[:, :, 0:64], ps[:].rearrange("p (h m) -> p h m", h=H)
                    )
                    nc.vector.tensor_copy(vv[:, :, 64:65], ones_sb[:, :, None])

                return group_thunks(KC, emit_mm, evac)

            def run(thunks):
                for t in thunks:
                    t()

            def attention(sl, hp, fillers=(), pops=(), default_pops=2,
                          pv_pre=None):
                fillers = list(fillers)
                h0, h1 = 2 * hp, 2 * hp + 1
                po0 = psA.tile([65, 512], F32, tag="acc", name="po0")
                po1 = psA.tile([65, 512], F32, tag="acc", name="po1")
                for tc_ in range(ST):
                    n = pops[tc_] if tc_ < len(pops) else default_pops
                    for _ in range(n):
                        if fillers:
                            fillers.pop(0)()
                    ksl, kcol = tc_ // 4, (tc_ % 4) * 128
                    pss = psS.tile([128, 1024], F32, tag="psS", name="psS")
                    nc.tensor.matmul(
                        pss[:, 0:512],
                        kT[hp][ksl][0:64, kcol:kcol + 128],
                        qT[hp][sl][0:64, :],
                        start=True, stop=True, tile_position=(0, 0),
                    )
                    nc.tensor.matmul(
                        pss[:, 512:1024],
                        kT[hp][ksl][64:128, kcol:kcol + 128],
                        qT[hp][sl][64:128, :],
                        start=True, stop=True, tile_position=(64, 0),
                    )
                    et = etp.tile([128, 1024], dt_mm, tag="et", name="et")
                    nc.scalar.activation(
                        et[:], pss[:], mybir.ActivationFunctionType.Exp
                    )
                    for t in (pv_pre or {}).get(tc_, ()):
                        t()
                    nc.tensor.matmul(
                        po0[:], v_sb[tc_][:, h0 * 65:(h0 + 1) * 65],
                        et[:, 0:512],
                        start=(tc_ == 0), stop=(tc_ == ST - 1),
                    )
                    nc.tensor.matmul(
                        po1[:], v_sb[tc_][:, h1 * 65:(h1 + 1) * 65],
                        et[:, 512:1024],
                        start=(tc_ == 0), stop=(tc_ == ST - 1),
                    )
                for half, po in ((0, po0), (1, po1)):
                    drow = normp.tile([1, 512], F32, tag="drow", name="drow")
                    nc.vector.tensor_copy(drow[:], po[64:65, :])
                    rrow = normp.tile([1, 512], F32, tag="rrow", name="rrow")
                    nc.vector.reciprocal_approx_fast(rrow[:], drow[:])
                    rbc = normp.tile([64, 512], F32, tag="rbc", name="rbc")
                    nc.gpsimd.partition_broadcast(rbc[:], rrow[:])
                    nc.vector.tensor_mul(
                        oT[hp][sl][half * 64:(half + 1) * 64, :],
                        po[0:64, :],
                        rbc[:],
                    )

            def op_thunks(sl, ct, order=None):
                order = list(order or range(IT))

                def emit_mm(ps, j):
                    ic = order[j]
                    nc.tensor.matmul(
                        ps[:],
                        wor[:, ic, ct * 128:(ct + 1) * 128],
                        oT[ic][sl][:],
                        start=(j == 0), stop=(j == IT - 1),
                    )

                def evac(ps):
                    ysb = outp.tile([128, 512], F32, tag="ysb", name="ysb")
                    nc.vector.tensor_scalar_add(ysb[:], ps[:],
                                                bo_sb[:, ct:ct + 1])
                    nc.sync.dma_start(
                        y_d[ct * 128:(ct + 1) * 128,
                            sl * 512:(sl + 1) * 512],
                        ysb[:],
                    )

                return group_thunks(IT, emit_mm, evac)

            y_part = [persist.tile([128, 512], F32, tag=f"yp{ct}",
                                   name=f"yp{ct}") for ct in range(IT)]

            def op_partial_thunks(ct):
                # ic 0..2 of the sl=1 projection, banked into SBUF (+bias)
                def emit_mm(ps, ic):
                    nc.tensor.matmul(
                        ps[:],
                        wor[:, ic, ct * 128:(ct + 1) * 128],
                        oT[ic][1][:],
                        start=(ic == 0), stop=(ic == IT - 2),
                    )

                def evac(ps):
                    nc.vector.tensor_scalar_add(y_part[ct][:], ps[:],
                                                bo_sb[:, ct:ct + 1])

                return group_thunks(IT - 1, emit_mm, evac)

            def op_final(ct):
                ps = psA.tile([128, 512], F32, tag="acc", name="acc")
                nc.tensor.matmul(
                    ps[:],
                    wor[:, IT - 1, ct * 128:(ct + 1) * 128],
                    oT[IT - 1][1][:],
                    start=True, stop=True,
                )
                ysb = outp.tile([128, 512], F32, tag="ysb", name="ysb")
                nc.vector.tensor_add(ysb[:], y_part[ct][:], ps[:])
                nc.sync.dma_start(
                    y_d[ct * 128:(ct + 1) * 128, 512:1024], ysb[:],
                )

            # ---- emission order (priority hint for the scheduler):
            # projections for head-pair hp+1 and the slab-0 output projection
            # are sprinkled between attention chunks so the PE fills the
            # slack of the exp-paced attention loop instead of monopolizing
            # it in blocks. ----
            # Emission = static scheduler priority. Fillers are single-matmul
            # thunks so the exp-paced attention loop is never blocked by a
            # multi-matmul projection block sitting ahead of the next scores
            # in the PE's strict-FIFO queue. Thunk order respects
            # write-before-read: qk pieces (q,sl0)=a (q,sl1)=b (k,sl0)=c
            # (k,sl1)=d; attention(0,hp) reads a,c at chunk 0 and d at
            # chunk 4; v_sb[t] is produced one chunk ahead of its PV.
            def qk4(hp):
                a = qk_thunks(wqr, bq_sb, qT, hp, 0)
                b = qk_thunks(wqr, bq_sb, qT, hp, 1)
                c = qk_thunks(wkr, bk_sb, kT, hp, 0)
                d = qk_thunks(wkr, bk_sb, kT, hp, 1)
                return a, b, c, d

            qk0 = qk4(0)
            qk1 = qk4(1)
            qk2 = qk4(2)
            qk3 = qk4(3)
            # prefix: only x/wq/wk-gated work, so the first scores + exp can
            # issue as soon as those land; v0 (gated on the later wv DMA) is
            # emitted between chunk 0's exp and its PV, and the v fillers pop
            # from chunk 1 on — nothing wv-gated ever sits ahead of ready
            # scores in the PE's strict-FIFO queue.
            run(qk0[0]); run(qk0[2])
            run(qk1[0]); run(qk1[2])
            vfill = []
            for t in (1, 2, 3):
                vfill += v_thunks(t)
            vfill += qk0[3]
            for t in (4, 5, 6, 7):
                vfill += v_thunks(t)
            attention(0, 0, vfill, pops=(0, 5, 5, 5, 5, 4, 4, 4),
                      pv_pre={0: v_thunks(0)})
            attention(0, 1, qk1[3] + qk1[1] + qk2[0] + qk2[2])
            attention(0, 2, qk2[3] + qk2[1] + qk3[0] + qk3[2])
            attention(0, 3, qk3[3] + qk3[1] + qk0[1])
            c0 = op_thunks(0, 0)
            c1 = op_thunks(0, 1)
            c2 = op_thunks(0, 2)
            c3 = op_thunks(0, 3)
            op0 = (c0[:3] + c1[:3] + [c0[3], c1[3]]
                   + c2[:3] + c3[:3] + [c2[3], c3[3]])
            attention(1, 0, op0, pops=(0, 0, 3, 3, 3, 3, 2, 2))
            attention(1, 1, [])
            attention(1, 2, [])
            op1p = op_partial_thunks(0) + op_partial_thunks(1) \
                + op_partial_thunks(2) + op_partial_thunks(3)
            attention(1, 3, op1p, pops=(0, 0, 0, 0, 3, 3, 3, 3))
            for ct in range(IT):
                op_final(ct)

    nc.compile()
    return nc


def prep_host(inputs, dt_mm):
    """Fold BN + scale + v-bias into effective weights (fp32 numpy)."""
    x = np.asarray(inputs["x"], dtype=np.float32)
    g = np.asarray(inputs["bn_gamma"], dtype=np.float32)
    be = np.asarray(inputs["bn_beta"], dtype=np.float32)
    mu = np.asarray(inputs["bn_mean"], dtype=np.float32)
    var = np.asarray(inputs["bn_var"], dtype=np.float32)
    wq = np.asarray(inputs["wq"], dtype=np.float32)
    bq = np.asarray(inputs["bq"], dtype=np.float32)
    wk = np.asarray(inputs["wk"], dtype=np.float32)
    bk = np.asarray(inputs["bk"], dtype=np.float32)
    wv = np.asarray(inputs["wv"], dtype=np.float32)
    bv = np.asarray(inputs["bv"], dtype=np.float32)
    wo = np.asarray(inputs["wo"], dtype=np.float32)
    bo = np.asarray(inputs["bo"], dtype=np.float32)

    a = g / np.sqrt(var + EPS)          # [C]
    bvec = be - mu * a                  # [C]

    wq_eff = wq * a[None, :] * SCALE
    bq_eff = (bq + wq @ bvec) * SCALE
    wk_eff = wk * a[None, :]
    bk_eff = bk + wk @ bvec
    wv_eff = wv * a[None, :]
    bv_eff = bv + wv @ bvec
    bo_eff = bo + wo @ bv_eff           # v bias rides through softmax (sums to 1)

    bias_pack = np.concatenate(
        [bq_eff.reshape(IT, 128).T, bk_eff.reshape(IT, 128).T,
         bo_eff.reshape(IT, 128).T], axis=1
    ).astype(np.float32)

    np_dt = np.float16 if mybir.dt.size(dt_mm) == 2 else np.float32

    def dev_layout(a):
        # [C_or_I, N] -> [128, KC, N]: partition p holds rows {k*128+p}
        return np.ascontiguousarray(
            a.reshape(KC, 128, a.shape[1]).transpose(1, 0, 2).astype(np_dt))

    wq_l = dev_layout(wq_eff.T)
    wk_l = dev_layout(wk_eff.T)
    wv_l = dev_layout(wv_eff.T)
    wo_l = dev_layout(wo.T)
    per_core = []
    for b in range(B):
        per_core.append({
            "x": dev_layout(x[b, :, :, 0]),
            "wqT": wq_l,
            "wkT": wk_l,
            "wvT": wv_l,
            "woT": wo_l,
            "bias_pack": np.ascontiguousarray(bias_pack),
        })
    return per_core


def _get_nc(dt_mm):
    key = str(dt_mm)
    if key not in _CACHE:
        _CACHE[key] = build_bass(dt_mm)
    return _CACHE[key]


def kernel(**inputs):
    nc = _get_nc(DT_MM)
    in_maps = prep_host(inputs, DT_MM)
    res = run_bass_kernel_spmd(nc, in_maps, list(range(N_CORES)))
    y = np.stack([res.results[c]["y"] for c in range(N_CORES)], axis=0)
    return y[..., None].astype(np.float32)


def run_traced(**inputs):
    """Like kernel() but with NTFF profiling; returns (y, results, tmpdir)."""
    nc = _get_nc(DT_MM)
    in_maps = prep_host(inputs, DT_MM)
    import tempfile
    tmpdir = tempfile.mkdtemp(prefix="mha_trace_")
    res = run_bass_kernel_spmd(
        nc, in_maps, list(range(N_CORES)), trace=True, tmpdir=tmpdir
    )
    y = np.stack([res.results[c]["y"] for c in range(N_CORES)], axis=0)
    return y[..., None].astype(np.float32), res, tmpdir


# revision 30
# speedup vs baseline: 1.0224x; 1.0224x over previous
"""Trainium2 Bass kernel for BatchNorm2d + 8-head self-attention block.

Reference (per batch element b, all fp32):
    xn = BN_eval(x[b]); t = xn.T
    q/k/v = t @ W.T + b            # [S, 512], 8 heads x 64
    attn  = softmax(q k^T / 8)     # per head
    y[b]  = ((attn v) @ wo.T + bo).T

Sharding: pure data parallel — one batch element per NeuronCore, weights
replicated, no collectives.

Device design (per core), fully in the "transposed" domain (no large
transposes anywhere):
  - BN folded into QKV weights/biases on host; 1/8 scale folded into wq/bq;
    v bias folded into bo (softmax rows sum to 1).
  - Q^T,K^T [I,S] = wT.T @ x      (x arrives [C,S] — natural rhs)
  - V [S,I]       = x_chunk.T @ wvT, stored interleaved per head with a
    ones column ([128, 8*65]) so the PV matmul (M=65) also produces the
    softmax denominators for free.
  - scores^T per head [t,s]; head pairs row-packed via tile_position
    (0,0)/(64,0), K=64 each; exp on ScalarE over both heads in one call
    (no max subtraction — scores are in [-3, 3]).
  - o^T accumulates over 8 t-chunks (K=128); normalize = approx-reciprocal
    row + gpsimd partition-broadcast + DVE multiply; y^T = woT.T @ o^T + bo.

Matmul dtype is fp16 by default: 2-byte weights keep LDWEIGHTS in the PE's
background buffer (hidden behind the previous matmul) where 4-byte fp32r
weights serialize ~150ns per matmul; fp16's 10-bit mantissa keeps the end
to-end error ~1e-3 of scale (all activations are within [-20, 20]).
Set dt_mm=float32r for a ~1e-4-accuracy variant (~1.4x slower).
"""

import numpy as np

import concourse.bass as bass
import concourse.tile as tile
from concourse import bacc, mybir
from concourse.bass_utils import run_bass_kernel_spmd
from concourse.tile import add_dep_helper

B, C, S = 8, 512, 1024
H, DH, INNER = 8, 64, 512
EPS = 1e-5
SCALE = DH ** (-0.5)
N_CORES = 8
F32 = mybir.dt.float32
F32R = mybir.dt.float32r
F16 = mybir.dt.float16

DT_MM = F16  # matmul dtype: F16 (fast) or F32R (precise)

_CACHE: dict = {}

KC = C // 128      # 4 contraction chunks over channels
IT = INNER // 128  # 4 tiles over inner dim (also head-pair index)
ST = S // 128      # 8 t-chunks
NSLAB = S // 512   # 2 s-slabs


def build_bass(dt_mm):
    two_byte = mybir.dt.size(dt_mm) == 2
    dt_in = dt_mm if two_byte else F32
    nc = bacc.Bacc("TRN2", target_bir_lowering=False, debug=False,
                   num_devices=N_CORES)

    # inputs arrive pre-arranged on the host into the SBUF layout
    # [partition, k-chunk, free] so every DMA is contiguous per partition
    x_d = nc.dram_tensor("x", [128, KC, S], dt_in, kind="ExternalInput")
    wqT_d = nc.dram_tensor("wqT", [128, KC, 512], dt_in, kind="ExternalInput")
    wkT_d = nc.dram_tensor("wkT", [128, KC, 512], dt_in, kind="ExternalInput")
    wvT_d = nc.dram_tensor("wvT", [128, KC, 512], dt_in, kind="ExternalInput")
    woT_d = nc.dram_tensor("woT", [128, KC, 512], dt_in, kind="ExternalInput")
    # bq | bk | bo packed on host as [128, 12] (col t+0/4/8 = vec[t*128+p])
    bias_d = nc.dram_tensor("bias_pack", [128, 3 * IT], F32, kind="ExternalInput")
    y_d = nc.dram_tensor("y", [C, S], F32, kind="ExternalOutput")

    with tile.TileContext(nc) as tc:
        with (
            tc.tile_pool(name="persist", bufs=1) as persist,
            tc.tile_pool(name="stage", bufs=2) as stage,
            tc.tile_pool(name="out", bufs=3) as outp,
            tc.tile_pool(name="et", bufs=6) as etp,
            tc.tile_pool(name="norm", bufs=2) as normp,
            # one shared 4-slot pool for every 1-bank accumulator (projection
            # groups AND the two attention po accumulators): a fresh bank is
            # always available at head-pair transitions, so the strict-FIFO
            # PE queue never stalls behind the normalize chain.
            tc.tile_pool(name="psA", bufs=4, space="PSUM") as psA,
            tc.tile_pool(name="psS", bufs=2, space="PSUM") as psS,
        ):
            # ---- loads, chained on the sync/HWDGE queue in need-order so
            # bandwidth goes to the critical transfer instead of round-robin;
            # tiny bias pack rides the gpsimd/SWDGE queue in parallel ----
            xr = persist.tile([128, KC, S], dt_mm, tag="xr", name="xr")
            wqr = persist.tile([128, KC, 512], dt_mm, tag="wqr", name="wqr")
            wkr = persist.tile([128, KC, 512], dt_mm, tag="wkr", name="wkr")
            wvr = persist.tile([128, KC, 512], dt_mm, tag="wvr", name="wvr")
            wor = persist.tile([128, KC, 512], dt_mm, tag="wor", name="wor")

            dmas = []
            if two_byte:
                # DMA straight into the matmul tiles — no casts needed.
                for h in range(2):
                    dmas.append(nc.sync.dma_start(
                        xr[:, 2 * h:2 * h + 2, :], x_d[:, 2 * h:2 * h + 2, :]))
                for dst, src in ((wqr, wqT_d), (wkr, wkT_d), (wvr, wvT_d),
                                 (wor, woT_d)):
                    dmas.append(nc.sync.dma_start(dst[:], src[:]))
            else:
                x_st = [stage.tile([128, 2, S], F32, tag=f"x_st{k}",
                                   name=f"x_st{k}", bufs=1) for k in range(2)]
                for h in range(2):
                    dmas.append(nc.sync.dma_start(
                        x_st[h][:], x_d[:, 2 * h:2 * h + 2, :]))
                w_st = {}
                for nm, src in (("wq", wqT_d), ("wk", wkT_d), ("wv", wvT_d),
                                ("wo", woT_d)):
                    st = stage.tile([128, KC, 512], F32, tag=f"{nm}_st",
                                    name=f"{nm}_st", bufs=1)
                    w_st[nm] = st
                    dmas.append(nc.sync.dma_start(st[:], src[:]))
            for a, b in zip(dmas[1:], dmas):
                add_dep_helper(a.ins, b.ins, sync=False, reason="dma priority")

            bias_sb = persist.tile([128, 3 * IT], F32, tag="bias")
            nc.gpsimd.dma_start(bias_sb[:], bias_d[:])
            bq_sb = bias_sb[:, 0:IT]
            bk_sb = bias_sb[:, IT:2 * IT]
            bo_sb = bias_sb[:, 2 * IT:3 * IT]

            if not two_byte:
                for kc in range(KC):
                    nc.vector.tensor_copy(xr[:, kc, :], x_st[kc // 2][:, kc % 2, :])
                nc.vector.tensor_copy(wqr[:], w_st["wq"][:])
                nc.vector.tensor_copy(wkr[:], w_st["wk"][:])
                nc.vector.tensor_copy(wvr[:], w_st["wv"][:])
                nc.vector.tensor_copy(wor[:], w_st["wo"][:])

            ones_sb = persist.tile([128, H], F32, tag="ones")
            nc.vector.memset(ones_sb[:], 1.0)

            # ---- PE warmup: ~4us of dummy matmuls during the initial DMA
            # wait so the HAM clock-gate reaches 8/8 before real work ----
            warm_sb = stage.tile([128, 256], dt_mm, tag="warm", bufs=1)
            nc.vector.memset(warm_sb[:], 0.0)
            warm_ps = psA.tile([128, 256], F32, tag="acc", name="warm_ps")
            for wi in range(34):
                nc.tensor.matmul(warm_ps[:], warm_sb[:, 0:128], warm_sb[:],
                                 start=(wi == 0), stop=(wi == 33))

            # ---- persistent per-slab outputs ----
            qT = [[persist.tile([128, 512], dt_mm, tag=f"qT{i}{s}",
                                name=f"qT{i}{s}") for s in range(NSLAB)]
                  for i in range(IT)]
            kT = [[persist.tile([128, 512], dt_mm, tag=f"kT{i}{s}",
                                name=f"kT{i}{s}") for s in range(NSLAB)]
                  for i in range(IT)]
            oT = [[persist.tile([128, 512], dt_mm, tag=f"oT{i}{s}",
                                name=f"oT{i}{s}") for s in range(NSLAB)]
                  for i in range(IT)]
            v_sb = [persist.tile([128, H * 65], dt_mm, tag=f"v{t}",
                                 name=f"v{t}") for t in range(ST)]

            def group_thunks(n_mm, emit_mm, evac):
                """n_mm single-matmul thunks accumulating into one psA bank;
                the first allocates the bank, the last appends the evacuation."""
                box = []

                def mk(i):
                    def t():
                        if i == 0:
                            box.append(psA.tile([128, 512], F32,
                                                tag="acc", name="acc"))
                        emit_mm(box[0], i)
                        if i == n_mm - 1:
                            evac(box[0])
                    return t

                return [mk(i) for i in range(n_mm)]

            def qk_thunks(w, bias, dst, hp, sl):
                def emit_mm(ps, kc):
                    nc.tensor.matmul(
                        ps[:],
                        w[:, kc, hp * 128:(hp + 1) * 128],
                        xr[:, kc, sl * 512:(sl + 1) * 512],
                        start=(kc == 0), stop=(kc == KC - 1),
                    )

                def evac(ps):
                    nc.vector.tensor_scalar_add(
                        dst[hp][sl][:], ps[:], bias[:, hp:hp + 1]
                    )

                return group_thunks(KC, emit_mm, evac)

            def v_thunks(tc_):
                def emit_mm(ps, kc):
                    nc.tensor.matmul(
                        ps[:],
                        xr[:, kc, tc_ * 128:(tc_ + 1) * 128],
                        wvr[:, kc, :],
                        start=(kc == 0), stop=(kc == KC - 1),
                    )

                def evac(ps):
                    vv = v_sb[tc_][:].rearrange("p (h m) -> p h m", h=H)
                    nc.vector.tensor_copy(
                        vv[:, :, 0:64], ps[:].rearrange("p (h m) -> p h m", h=H)
                    )
                    nc.vector.tensor_copy(vv[:, :, 64:65], ones_sb[:, :, None])

                return group_thunks(KC, emit_mm, evac)

            def run(thunks):
                for t in thunks:
                    t()

            def attention(sl, hp, fillers=(), pops=(), default_pops=2,
                          pv_pre=None):
                fillers = list(fillers)
                h0, h1 = 2 * hp, 2 * hp + 1
                po0 = psA.tile([65, 512], F32, tag="acc", name="po0")
                po1 = psA.tile([65, 512], F32, tag="acc", name="po1")
                for tc_ in range(ST):
                    n = pops[tc_] if tc_ < len(pops) else default_pops
                    for _ in range(n):
                        if fillers:
                            fillers.pop(0)()
                    ksl, kcol = tc_ // 4, (tc_ % 4) * 128
                    pss = psS.tile([128, 1024], F32, tag="psS", name="psS")
                    nc.tensor.matmul(
                        pss[:, 0:512],
                        kT[hp][ksl][0:64, kcol:kcol + 128],
                        qT[hp][sl][0:64, :],
                        start=True, stop=True, tile_position=(0, 0),
                    )
                    nc.tensor.matmul(
                        pss[:, 512:1024],
                        kT[hp][ksl][64:128, kcol:kcol + 128],
                        qT[hp][sl][64:128, :],
                        start=True, stop=True, tile_position=(64, 0),
                    )
                    et = etp.tile([128, 1024], dt_mm, tag="et", name="et")
                    nc.scalar.activation(
                        et[:], pss[:], mybir.ActivationFunctionType.Exp
                    )
                    for t in (pv_pre or {}).get(tc_, ()):
                        t()
                    nc.tensor.matmul(
                        po0[:], v_sb[tc_][:, h0 * 65:(h0 + 1) * 65],
                        et[:, 0:512],
                        start=(tc_ == 0), stop=(tc_ == ST - 1),
                    )
                    nc.tensor.matmul(
                        po1[:], v_sb[tc_][:, h1 * 65:(h1 + 1) * 65],
                        et[:, 512:1024],
                        start=(tc_ == 0), stop=(tc_ == ST - 1),
                    )
                for half, po in ((0, po0), (1, po1)):
                    drow = normp.tile([1, 512], F32, tag="drow", name="drow")
                    nc.vector.tensor_copy(drow[:], po[64:65, :])
                    rrow = normp.tile([1, 512], F32, tag="rrow", name="rrow")
                    nc.vector.reciprocal_approx_fast(rrow[:], drow[:])
                    rbc = normp.tile([64, 512], F32, tag="rbc", name="rbc")
                    nc.gpsimd.partition_broadcast(rbc[:], rrow[:])
                    nc.vector.tensor_mul(
                        oT[hp][sl][half * 64:(half + 1) * 64, :],
                        po[0:64, :],
                        rbc[:],
                    )

            def op_thunks(sl, ct, order=None):
                order = list(order or range(IT))

                def emit_mm(ps, j):
                    ic = order[j]
                    nc.tensor.matmul(
                        ps[:],
                        wor[:, ic, ct * 128:(ct + 1) * 128],
                        oT[ic][sl][:],
                        start=(j == 0), stop=(j == IT - 1),
                    )

                def evac(ps):
                    ysb = outp.tile([128, 512], F32, tag="ysb", name="ysb")
                    nc.vector.tensor_scalar_add(ysb[:], ps[:],
                                                bo_sb[:, ct:ct + 1])
                    nc.sync.dma_start(
                        y_d[ct * 128:(ct + 1) * 128,
                            sl * 512:(sl + 1) * 512],
                        ysb[:],
                    )

                return group_thunks(IT, emit_mm, evac)

            y_part = [persist.tile([128, 512], F32, tag=f"yp{ct}",
                                   name=f"yp{ct}") for ct in range(IT)]

            def op_partial_thunks(ct):
                # ic 0..2 of the sl=1 projection, banked into SBUF (+bias)
                def emit_mm(ps, ic):
                    nc.tensor.matmul(
                        ps[:],
                        wor[:, ic, ct * 128:(ct + 1) * 128],
                        oT[ic][1][:],
                        start=(ic == 0), stop=(ic == IT - 2),
                    )

                def evac(ps):
                    nc.vector.tensor_scalar_add(y_part[ct][:], ps[:],
                                                bo_sb[:, ct:ct + 1])

                return group_thunks(IT - 1, emit_mm, evac)

            def op_final(ct):
                ps = psA.tile([128, 512], F32, tag="acc", name="acc")
                nc.tensor.matmul(
                    ps[:],
                    wor[:, IT - 1, ct * 128:(ct + 1) * 128],
                    oT[IT - 1][1][:],
                    start=True, stop=True,
                )
                ysb = outp.tile([128, 512], F32, tag="ysb", name="ysb")
                nc.vector.tensor_add(ysb[:], y_part[ct][:], ps[:])
                nc.sync.dma_start(
                    y_d[ct * 128:(ct + 1) * 128, 512:1024], ysb[:],
                )

            # ---- emission order (priority hint for the scheduler):
            # projections for head-pair hp+1 and the slab-0 output projection
            # are sprinkled between attention chunks so the PE fills the
            # slack of the exp-paced attention loop instead of monopolizing
            # it in blocks. ----
            # Emission = static scheduler priority. Fillers are single-matmul
            # thunks so the exp-paced attention loop is never blocked by a
            # multi-matmul projection block sitting ahead of the next scores
            # in the PE's strict-FIFO queue. Thunk order respects
            # write-before-read: qk pieces (q,sl0)=a (q,sl1)=b (k,sl0)=c
            # (k,sl1)=d; attention(0,hp) reads a,c at chunk 0 and d at
            # chunk 4; v_sb[t] is produced one chunk ahead of its PV.
            def qk4(hp):
                a = qk_thunks(wqr, bq_sb, qT, hp, 0)
                b = qk_thunks(wqr, bq_sb, qT, hp, 1)
                c = qk_thunks(wkr, bk_sb, kT, hp, 0)
                d = qk_thunks(wkr, bk_sb, kT, hp, 1)
                return a, b, c, d

            qk0 = qk4(0)
            qk1 = qk4(1)
            qk2 = qk4(2)
            qk3 = qk4(3)
            run(qk0[0]); run(qk0[2]); run(v_thunks(0))
            run(qk1[0]); run(qk1[2])
            # v_sb[t] lands one chunk before its PV; kT[0][1] (qk0 d) must
            # land by chunk 4 — popped at chunk 3.
            vfill = []
            for t in (1, 2, 3):
                vfill += v_thunks(t)
            vfill += qk0[3]
            for t in (4, 5, 6, 7):
                vfill += v_thunks(t)
            attention(0, 0, vfill, pops=(4, 4, 4, 4, 4, 4, 4, 4))
            attention(0, 1, qk1[3] + qk1[1] + qk2[0] + qk2[2])
            attention(0, 2, qk2[3] + qk2[1] + qk3[0] + qk3[2])
            attention(0, 3, qk3[3] + qk3[1] + qk0[1])
            c0 = op_thunks(0, 0)
            c1 = op_thunks(0, 1)
            c2 = op_thunks(0, 2)
            c3 = op_thunks(0, 3)
            op0 = (c0[:3] + c1[:3] + [c0[3], c1[3]]
                   + c2[:3] + c3[:3] + [c2[3], c3[3]])
            attention(1, 0, op0, pops=(0, 0, 3, 3, 3, 3, 2, 2))
            attention(1, 1, [])
            attention(1, 2, [])
            op1p = op_partial_thunks(0) + op_partial_thunks(1) \
                + op_partial_thunks(2) + op_partial_thunks(3)
            attention(1, 3, op1p, pops=(0, 0, 0, 0, 3, 3, 3, 3))
            for ct in range(IT):
                op_final(ct)

    nc.compile()
    return nc


def prep_host(inputs, dt_mm):
    """Fold BN + scale + v-bias into effective weights (fp32 numpy)."""
    x = np.asarray(inputs["x"], dtype=np.float32)
    g = np.asarray(inputs["bn_gamma"], dtype=np.float32)
    be = np.asarray(inputs["bn_beta"], dtype=np.float32)
    mu = np.asarray(inputs["bn_mean"], dtype=np.float32)
    var = np.asarray(inputs["bn_var"], dtype=np.float32)
    wq = np.asarray(inputs["wq"], dtype=np.float32)
    bq = np.asarray(inputs["bq"], dtype=np.float32)
    wk = np.asarray(inputs["wk"], dtype=np.float32)
    bk = np.asarray(inputs["bk"], dtype=np.float32)
    wv = np.asarray(inputs["wv"], dtype=np.float32)
    bv = np.asarray(inputs["bv"], dtype=np.float32)
    wo = np.asarray(inputs["wo"], dtype=np.float32)
    bo = np.asarray(inputs["bo"], dtype=np.float32)

    a = g / np.sqrt(var + EPS)          # [C]
    bvec = be - mu * a                  # [C]

    wq_eff = wq * a[None, :] * SCALE
    bq_eff = (bq + wq @ bvec) * SCALE
    wk_eff = wk * a[None, :]
    bk_eff = bk + wk @ bvec
    wv_eff = wv * a[None, :]
    bv_eff = bv + wv @ bvec
    bo_eff = bo + wo @ bv_eff           # v bias rides through softmax (sums to 1)

    bias_pack = np.concatenate(
        [bq_eff.reshape(IT, 128).T, bk_eff.reshape(IT, 128).T,
         bo_eff.reshape(IT, 128).T], axis=1
    ).astype(np.float32)

    np_dt = np.float16 if mybir.dt.size(dt_mm) == 2 else np.float32

    def dev_layout(a):
        # [C_or_I, N] -> [128, KC, N]: partition p holds rows {k*128+p}
        return np.ascontiguousarray(
            a.reshape(KC, 128, a.shape[1]).transpose(1, 0, 2).astype(np_dt))

    wq_l = dev_layout(wq_eff.T)
    wk_l = dev_layout(wk_eff.T)
    wv_l = dev_layout(wv_eff.T)
    wo_l = dev_layout(wo.T)
    per_core = []
    for b in range(B):
        per_core.append({
            "x": dev_layout(x[b, :, :, 0]),
            "wqT": wq_l,
            "wkT": wk_l,
            "wvT": wv_l,
            "woT": wo_l,
            "bias_pack": np.ascontiguousarray(bias_pack),
        })
    return per_core


def _get_nc(dt_mm):
    key = str(dt_mm)
    if key not in _CACHE:
        _CACHE[key] = build_bass(dt_mm)
    return _CACHE[key]


def kernel(**inputs):
    nc = _get_nc(DT_MM)
    in_maps = prep_host(inputs, DT_MM)
    res = run_bass_kernel_spmd(nc, in_maps, list(range(N_CORES)))
    y = np.stack([res.results[c]["y"] for c in range(N_CORES)], axis=0)
    return y[..., None].astype(np.float32)


def run_traced(**inputs):
    """Like kernel() but with NTFF profiling; returns (y, results, tmpdir)."""
    nc = _get_nc(DT_MM)
    in_maps = prep_host(inputs, DT_MM)
    import tempfile
    tmpdir = tempfile.mkdtemp(prefix="mha_trace_")
    res = run_bass_kernel_spmd(
        nc, in_maps, list(range(N_CORES)), trace=True, tmpdir=tmpdir
    )
    y = np.stack([res.results[c]["y"] for c in range(N_CORES)], axis=0)
    return y[..., None].astype(np.float32), res, tmpdir
